# revision 2
# baseline (speedup 1.0000x reference)
"""MoE top-1 feed-forward (DeepSpeed-style) on 8 Trainium2 NeuronCores.

Strategy (expert parallelism, per the sharding hint):
  - Host computes the (tiny) gate: logits = x @ Wg, softmax, top-1 expert id
    and gate prob per token (float64 for a faithful argmax).
  - Tokens are dispatched to the core owning their expert (core e holds
    W1[e]/b1[e]/W2[e]/b2[e]); each core's token batch is padded to a common
    capacity C so all 8 cores run one SPMD program.
  - Each core runs the dense FFN for its tokens:
        hT = silu(W1^T @ xT + b1);  yT = W2^T @ hT
    with tokens laid out along the free (moving) dimension so no transposes
    are needed on device: xT is [D, C], hT is [F, C], yT is [D, C].
  - All images are host-packed to the EXACT SBUF layout so every DMA is
    contiguous per partition row (few, large descriptors; the descriptor
    storm of a strided gather was the dominant head/tail latency).
  - bf16 weights/activations (f32 PSUM accumulate): halves HBM traffic vs
    f32 and runs the PE at full rate; the problem sits at the roofline
    ridge (~31us HBM, ~33us PE per core).
  - Rings: sync=x+W1(+y), gpsimd=W2+b1(+y), scalar=silu+evac, vector=
    fanout+evac, tensor=matmul only.  A short junk-matmul warmup ramps the
    PE p-state (0.65->2.4GHz) while the first loads land.
  - Host combines: out[token] = gate * (y + b2[expert]).
"""

import os
import sys

import numpy as np

try:
    import concourse.mybir as mybir  # noqa: F401
except ModuleNotFoundError:  # fallback if the site hooks aren't installed
    sys.path.insert(0, "/opt/trn_rl_repo")

import concourse.mybir as mybir
import concourse.tile as tile
from concourse import bacc
from concourse.bass_utils import run_bass_kernel_spmd

N_CORES = 8

# Compute dtype for the matmuls:
#   "bf16" - weights/activations cast to bfloat16 (f32 PSUM accumulate)
#   "f32r" - fp32 data, PE's replicated-fp32 mode (full rate at N>=256)
#   "f32"  - plain fp32 matmuls (4x slower PE)
MODE = os.environ.get("BASS_MOE_MODE", "bf16")

FG = int(os.environ.get("BASS_MOE_FG", "4"))  # steady-state f-chunks per W1 group
W2P = int(os.environ.get("BASS_MOE_W2P", "2"))  # f-chunks per W2 pair-tile
N_WARM = int(os.environ.get("BASS_MOE_WARM", "10"))  # PE p-state warmup matmuls
W2PRIME = int(os.environ.get("BASS_MOE_W2PRIME", "3"))  # W2 tiles loaded up front


def _w1_groups(KF):
    """F-chunk widths per W1 group: small leading groups let the PE start
    before a whole FG-wide image lands."""
    lead = [1, 1, 2] if FG > 2 and KF > 8 else []
    rem = KF - sum(lead)
    groups = list(lead)
    while rem > 0:
        w = min(FG, rem)
        groups.append(w)
        rem -= w
    return groups


_CACHE: dict = {}
_PACK_CACHE: dict = {}


def _roundup(a: int, m: int) -> int:
    return -(-a // m) * m


def _build_bass(C: int, n_slabs: int, mode: str, D: int, F: int):
    """Build + compile the per-core Bass program for capacity C (divisible by
    n_slabs; slab width CS = C/n_slabs must be 256..512)."""
    f32 = mybir.dt.float32
    if mode == "bf16":
        dt_io = mybir.dt.bfloat16
    elif mode == "f32r":
        dt_io = mybir.dt.float32r
    else:
        dt_io = f32

    KD, KF = D // 128, F // 128
    GRPS = _w1_groups(KF)
    NP = KF // W2P  # number of W2 pair-tiles
    CS = C // n_slabs
    assert C % n_slabs == 0 and 256 <= CS <= 512

    nc = bacc.Bacc(None, target_bir_lowering=False, debug=False)
    # Host-packed images (see kernel() for the packing); every load/store is
    # contiguous per partition row:
    #   xX   [n_slabs, 128, KD*CS]  xX[s,p,d*CS+c] = x[s*CS+c, d*128+p]
    #   w1   [128, KD*F]            flat group images; group g at column
    #                               offset KD*128*sum(GRPS[:g]), blocks (d, j)
    #                               within a group at (d*gw+j)*128
    #   w2   [NP, 128, W2P*D]       w2[p] f-chunk r=f-p*W2P at cols r*D
    #   b1r  [128, KF]              b1[f*128+p] at [p, f]
    #   yX   [n_slabs, 128, KD*CS]  output, same layout as xX (dt_io)
    xX = nc.dram_tensor("xX", [n_slabs, 128, KD * CS], dt_io, kind="ExternalInput")
    w1 = nc.dram_tensor("w1", [128, KD * F], dt_io, kind="ExternalInput")
    w2 = nc.dram_tensor("w2", [NP, 128, W2P * D], dt_io, kind="ExternalInput")
    b1r = nc.dram_tensor("b1r", [128, KF], f32, kind="ExternalInput")
    yX = nc.dram_tensor("yX", [n_slabs, 128, KD * CS], dt_io, kind="ExternalOutput")

    silu = mybir.ActivationFunctionType.Silu

    with tile.TileContext(nc) as tc:
        with (
            tc.tile_pool(name="xp", bufs=2) as xp,
            tc.tile_pool(name="w1p", bufs=4) as w1p,
            tc.tile_pool(name="w2p", bufs=1) as w2p,
            tc.tile_pool(name="hp", bufs=4) as hp,
            tc.tile_pool(name="bp", bufs=1) as bp,
            tc.tile_pool(name="yp", bufs=2) as yp,
            tc.tile_pool(name="jp", bufs=1) as jp,
            tc.tile_pool(name="ps_h", bufs=2, space="PSUM") as ps_h,
            tc.tile_pool(name="ps_y", bufs=1, space="PSUM") as ps_y,
        ):
            w2ts: list = [None] * NP

            def load_w2(p):
                t = w2p.tile([128, W2P * D], dt_io, tag=f"w2_{p}", name=f"w2t{p}")
                nc.gpsimd.dma_start(out=t[:], in_=w2[p])
                w2ts[p] = t

            # PE p-state warmup: junk matmuls with no data deps keep the PE
            # busy from ~t=0 so the 0.65/1.2GHz ramp is spent while the first
            # weight/activation DMAs land, not on real work.
            junk = jp.tile([128, CS], dt_io, tag="junk", name="junk")
            nc.vector.memset(junk[:], 0)
            for i in range(N_WARM):
                pj = ps_h.tile([128, CS], f32, tag="hps", name="pjunk")
                nc.tensor.matmul(
                    pj[:], junk[:, 0:128], junk[:], start=True, stop=True
                )

            b1_done = False
            for s in range(n_slabs):
                # one wide contiguous x DMA, fanned out to narrow per-d tiles
                # on the vector engine; narrow rhs tiles keep the PE
                # moving-operand read on its fast path
                xw = xp.tile([128, KD * CS], dt_io, tag="xw", name="xw")
                nc.sync.dma_start(out=xw[:], in_=xX[s])
                if not b1_done:
                    b1t = bp.tile([128, KF], f32, tag="b1", name="b1t")
                    nc.gpsimd.dma_start(out=b1t[:], in_=b1r[:])
                    b1_done = True
                    for p in range(min(NP, W2PRIME)):
                        load_w2(p)
                xt = []
                for d in range(KD):
                    t = xp.tile([128, CS], dt_io, tag=f"x{d}", name=f"xt{d}")
                    nc.vector.tensor_copy(t[:], xw[:, d * CS : (d + 1) * CS])
                    xt.append(t)
                py = [
                    ps_y.tile([128, CS], f32, tag=f"y{dd}", name=f"py{dd}")
                    for dd in range(KD)
                ]

                def emit_mm2(f, ht):
                    # yT += W2[f-chunk, :]^T @ hT[f-chunk]
                    p, r = divmod(f, W2P)
                    for dd in range(KD):
                        nc.tensor.matmul(
                            py[dd][:],
                            w2ts[p][:, r * D + dd * 128 : r * D + (dd + 1) * 128],
                            ht[:],
                            start=(f == 0),
                            stop=(f == KF - 1),
                        )

                pend = None  # (f, ht) whose mm2 is deferred one chunk
                f0 = 0
                for g, gw in enumerate(GRPS):
                    off = KD * 128 * f0
                    w1g = w1p.tile(
                        [128, KD * gw * 128],
                        dt_io,
                        tag="w1g",
                        name=f"w1g{g}",
                        padded_shape=[128, KD * FG * 128],
                    )
                    nc.sync.dma_start(
                        out=w1g[:], in_=w1[:, off : off + KD * gw * 128]
                    )
                    for j in range(gw):
                        f = f0 + j
                        # hT[f-chunk] = silu(sum_d W1[d, f-chunk]^T @ xT[d] + b1)
                        ph = ps_h.tile([128, CS], f32, tag="hps", name="ph")
                        for d in range(KD):
                            nc.tensor.matmul(
                                ph[:],
                                w1g[:, (d * gw + j) * 128 : (d * gw + j + 1) * 128],
                                xt[d][:],
                                start=(d == 0),
                                stop=(d == KD - 1),
                            )
                        ht = hp.tile([128, CS], dt_io, tag="ht", name="ht")
                        nc.scalar.activation(ht[:], ph[:], silu, bias=b1t[:, f : f + 1])
                        # keep W2 streaming ~two tiles ahead on the gpsimd ring
                        if s == 0 and f % W2P == W2P - 1:
                            nxt = (f + 1) // W2P + W2PRIME - 1
                            if nxt < NP:
                                load_w2(nxt)
                        # mm2 for the PREVIOUS chunk: its silu ran while this
                        # chunk's mm1 was on the PE, so the PE never waits on
                        # the ACT engine
                        if pend is not None:
                            emit_mm2(*pend)
                        pend = (f, ht)
                    f0 += gw
                if pend is not None:
                    emit_mm2(*pend)
                    pend = None

                # tail: evacuate PSUM on both DVE (dd<3) and ACT (dd>=3),
                # casting to dt_io, then stream out one contiguous half per
                # idle ring (sync / gpsimd)
                yt = yp.tile([128, KD * CS], dt_io, tag="yt", name="yt")
                half = KD // 2
                for dd in range(KD):
                    if dd < half:
                        nc.vector.tensor_copy(
                            yt[:, dd * CS : (dd + 1) * CS], py[dd][:]
                        )
                    else:
                        nc.scalar.copy(yt[:, dd * CS : (dd + 1) * CS], py[dd][:])
                nc.sync.dma_start(
                    out=yX[s][:, 0 : half * CS], in_=yt[:, 0 : half * CS]
                )
                nc.gpsimd.dma_start(
                    out=yX[s][:, half * CS :], in_=yt[:, half * CS :]
                )

    nc.compile()
    return nc


def _get_bass(C: int, n_slabs: int, mode: str, D: int, F: int):
    key = (C, n_slabs, mode, D, F, FG, W2P, N_WARM, W2PRIME)
    if key not in _CACHE:
        _CACHE[key] = _build_bass(C, n_slabs, mode, D, F)
    return _CACHE[key]


def _gate_host(x: np.ndarray, Wg: np.ndarray):
    """Top-1 gating in float64: returns (expert_idx [T], gate [T] f32)."""
    logits = x.astype(np.float64) @ Wg.astype(np.float64)
    m = logits.max(-1, keepdims=True)
    p = np.exp(logits - m)
    p /= p.sum(-1, keepdims=True)
    return p.argmax(-1), p.max(-1).astype(np.float32)


def _kernel_numpy(x, Wg, W1, b1, W2, b2):
    """Reference-equivalent fallback (host only)."""
    idx, gate = _gate_host(x, Wg)
    out = np.zeros_like(x)
    for e in range(W1.shape[0]):
        ids = np.nonzero(idx == e)[0]
        if ids.size == 0:
            continue
        h = x[ids] @ W1[e] + b1[e]
        h = h * (1.0 / (1.0 + np.exp(-h)))
        out[ids] = gate[ids, None] * (h @ W2[e] + b2[e])
    return out


def _pack_weights(W1, b1, W2, np_io):
    """Per-expert weight images (cached across calls: weights don't change)."""
    key = (W1.ctypes.data, W2.ctypes.data, W1.shape, W2.shape, np_io)
    if key in _PACK_CACHE:
        return _PACK_CACHE[key]
    E, D, F = W1.shape
    KD, KF = D // 128, F // 128
    NP = KF // W2P
    grps = _w1_groups(KF)
    packed = []
    for e in range(E):
        w1e = W1[e].reshape(KD, 128, KF, 128)
        parts = []
        f0 = 0
        for gw in grps:
            blk = w1e[:, :, f0 : f0 + gw]  # [KD, 128, gw, 128]
            parts.append(blk.transpose(1, 0, 2, 3).reshape(128, KD * gw * 128))
            f0 += gw
        w1r = np.concatenate(parts, axis=1)  # [128, KD*F]
        w2r = (
            W2[e]
            .reshape(NP, W2P, 128, D)
            .transpose(0, 2, 1, 3)
            .reshape(NP, 128, W2P * D)
        )
        packed.append(
            {
                "w1": np.ascontiguousarray(w1r).astype(np_io, copy=False),
                "w2": np.ascontiguousarray(w2r).astype(np_io, copy=False),
                "b1r": np.ascontiguousarray(b1[e].reshape(KF, 128).T),
            }
        )
    _PACK_CACHE[key] = packed
    return packed


def kernel(hidden_states, Wg, W1, b1, W2, b2):
    hidden_states = np.asarray(hidden_states)
    Wg = np.asarray(Wg, dtype=np.float32)
    W1 = np.asarray(W1, dtype=np.float32)
    b1 = np.asarray(b1, dtype=np.float32)
    W2 = np.asarray(W2, dtype=np.float32)
    b2 = np.asarray(b2, dtype=np.float32)

    orig_shape = hidden_states.shape
    D = orig_shape[-1]
    x = np.ascontiguousarray(hidden_states, dtype=np.float32).reshape(-1, D)
    E, _, F = W1.shape
    KD, KF = D // 128, F // 128

    if E != N_CORES or D % 128 != 0 or F % 128 != 0 or KF % W2P != 0:
        return _kernel_numpy(x, Wg, W1, b1, W2, b2).reshape(orig_shape)

    idx, gate = _gate_host(x, Wg)
    order = np.argsort(idx, kind="stable")
    counts = np.bincount(idx, minlength=E)
    starts = np.concatenate([[0], np.cumsum(counts)])

    # Capacity: common padded token count per core. Slab width must be
    # 256..512 (PSUM bank limit / fp32r fast path).
    C = max(256, _roundup(int(counts.max()), 16))
    n_slabs = -(-C // 512)
    C = n_slabs * max(256, _roundup(-(-C // n_slabs), 16))
    CS = C // n_slabs

    mode = MODE
    np_io = np.float32
    if mode == "bf16":
        import ml_dtypes

        np_io = ml_dtypes.bfloat16

    nc = _get_bass(C, n_slabs, mode, D, F)

    wpacked = _pack_weights(W1, b1, W2, np_io)
    in_maps = []
    for e in range(E):
        ids = order[starts[e] : starts[e + 1]]
        xe = np.zeros((C, D), dtype=np.float32)
        xe[: ids.size] = x[ids]
        # [n_slabs, 128, KD*CS] SBUF image (contiguous per partition row)
        xr = xe.reshape(n_slabs, CS, KD, 128).transpose(0, 3, 2, 1)
        xr = np.ascontiguousarray(xr.reshape(n_slabs, 128, KD * CS))
        in_maps.append({"xX": xr.astype(np_io, copy=False), **wpacked[e]})

    res = run_bass_kernel_spmd(nc, in_maps, list(range(N_CORES)))

    out = np.zeros_like(x)
    for e in range(E):
        ids = order[starts[e] : starts[e + 1]]
        if ids.size == 0:
            continue
        yr = np.asarray(res.results[e]["yX"], dtype=np.float32)
        y = yr.reshape(n_slabs, 128, KD, CS).transpose(0, 3, 2, 1).reshape(C, D)
        out[ids] = gate[ids, None] * (y[: ids.size] + b2[e])
    return out.reshape(orig_shape)


# revision 6
# speedup vs baseline: 1.0931x; 1.0931x over previous
"""MoE top-1 feed-forward (DeepSpeed-style) on 8 Trainium2 NeuronCores.

Strategy (expert parallelism, per the sharding hint):
  - Host computes the (tiny) gate: logits = x @ Wg, softmax, top-1 expert id
    and gate prob per token (float64 for a faithful argmax).
  - Tokens are dispatched to the core owning their expert (core e holds
    W1[e]/b1[e]/W2[e]/b2[e]); each core's token batch is padded to a common
    capacity C so all 8 cores run one SPMD program.
  - Each core runs the dense FFN for its tokens:
        hT = silu(W1^T @ xT + b1);  yT = W2^T @ hT
    with tokens laid out along the free (moving) dimension so no transposes
    are needed on device: xT is [D, C], hT is [F, C], yT is [D, C].
  - All images are host-packed to the EXACT SBUF layout so every DMA is
    contiguous per partition row (few, large descriptors; the descriptor
    storm of a strided gather was the dominant head/tail latency).
  - bf16 weights/activations (f32 PSUM accumulate): halves HBM traffic vs
    f32 and runs the PE at full rate; the problem sits at the roofline
    ridge (~31us HBM, ~33us PE per core).
  - Rings: sync=x+W1(+y), gpsimd=W2+b1(+y), scalar=silu+evac, vector=
    fanout+evac, tensor=matmul only.  A short junk-matmul warmup ramps the
    PE p-state (0.65->2.4GHz) while the first loads land.
  - Host combines: out[token] = gate * (y + b2[expert]).
"""

import os
import sys

import numpy as np

try:
    import concourse.mybir as mybir  # noqa: F401
except ModuleNotFoundError:  # fallback if the site hooks aren't installed
    sys.path.insert(0, "/opt/trn_rl_repo")

import concourse.mybir as mybir
import concourse.tile as tile
from concourse import bacc
from concourse.bass_utils import run_bass_kernel_spmd

N_CORES = 8

# Compute dtype for the matmuls:
#   "bf16" - weights/activations cast to bfloat16 (f32 PSUM accumulate)
#   "f32r" - fp32 data, PE's replicated-fp32 mode (full rate at N>=256)
#   "f32"  - plain fp32 matmuls (4x slower PE)
MODE = os.environ.get("BASS_MOE_MODE", "bf16")

FG = int(os.environ.get("BASS_MOE_FG", "2"))  # steady-state f-chunks per W1 group
W2P = int(os.environ.get("BASS_MOE_W2P", "2"))  # f-chunks per W2 pair-tile
N_WARM = int(os.environ.get("BASS_MOE_WARM", "11"))  # PE p-state warmup matmuls


def _w1_groups(KF):
    """F-chunk widths per W1 group: small leading groups let the PE start
    before a whole FG-wide image lands."""
    lead = [1, 1] if FG > 1 and KF > 8 else []
    rem = KF - sum(lead)
    groups = list(lead)
    while rem > 0:
        w = min(FG, rem)
        groups.append(w)
        rem -= w
    return groups


_CACHE: dict = {}
_PACK_CACHE: dict = {}


def _roundup(a: int, m: int) -> int:
    return -(-a // m) * m


def _build_bass(C: int, n_slabs: int, mode: str, D: int, F: int):
    """Build + compile the per-core Bass program for capacity C (divisible by
    n_slabs; slab width CS = C/n_slabs must be 256..512)."""
    f32 = mybir.dt.float32
    if mode == "bf16":
        dt_io = mybir.dt.bfloat16
    elif mode == "f32r":
        dt_io = mybir.dt.float32r
    else:
        dt_io = f32

    KD, KF = D // 128, F // 128
    GRPS = _w1_groups(KF)
    NP = KF // W2P  # number of W2 pair-tiles
    CS = C // n_slabs
    assert C % n_slabs == 0 and 256 <= CS <= 512

    nc = bacc.Bacc(None, target_bir_lowering=False, debug=False)
    # Host-packed images (see kernel() for the packing); every load/store is
    # contiguous per partition row:
    #   xX   [n_slabs, 128, KD*CS]  xX[s,p,d*CS+c] = x[s*CS+c, d*128+p]
    #   w1   [128, KD*F]            flat group images; group g at column
    #                               offset KD*128*sum(GRPS[:g]), blocks (d, j)
    #                               within a group at (d*gw+j)*128
    #   w2   [NP, 128, W2P*D]       w2[p] f-chunk r=f-p*W2P at cols r*D
    #   b1r  [128, KF]              b1[f*128+p] at [p, f]
    #   yX   [n_slabs, 128, KD*CS]  output, same layout as xX (dt_io)
    xX = nc.dram_tensor("xX", [n_slabs, 128, KD * CS], dt_io, kind="ExternalInput")
    w1 = nc.dram_tensor("w1", [128, KD * F], dt_io, kind="ExternalInput")
    w2 = nc.dram_tensor("w2", [NP, 128, W2P * D], dt_io, kind="ExternalInput")
    b1r = nc.dram_tensor("b1r", [128, KF], f32, kind="ExternalInput")
    yX = nc.dram_tensor("yX", [n_slabs, 128, KD * CS], dt_io, kind="ExternalOutput")

    silu = mybir.ActivationFunctionType.Silu

    with tile.TileContext(nc) as tc:
        with (
            tc.tile_pool(name="xp", bufs=2) as xp,
            tc.tile_pool(name="w1p", bufs=1) as w1p,
            tc.tile_pool(name="w2p", bufs=1) as w2p,
            tc.tile_pool(name="hp", bufs=4) as hp,
            tc.tile_pool(name="bp", bufs=1) as bp,
            tc.tile_pool(name="yp", bufs=2) as yp,
            tc.tile_pool(name="jp", bufs=1) as jp,
            tc.tile_pool(name="ps_h", bufs=2, space="PSUM") as ps_h,
            tc.tile_pool(name="ps_y", bufs=1, space="PSUM") as ps_y,
        ):
            # PE p-state warmup: junk matmuls with no data deps keep the PE
            # busy from the end of the preamble so the 0.65/1.2GHz clock ramp
            # is spent while the first weight/activation DMAs land, not on
            # real work.  gpsimd exits the preamble earliest and is idle.
            junk = jp.tile([128, CS], dt_io, tag="junk", name="junk")
            nc.gpsimd.memset(junk[:], 0)
            for i in range(N_WARM):
                pj = ps_h.tile([128, CS], f32, tag="hps", name="pjunk")
                nc.tensor.matmul(
                    pj[:], junk[:, 0:128], junk[:], start=True, stop=True
                )

            b1t = bp.tile([128, KF], f32, tag="b1", name="b1t")
            nc.gpsimd.dma_start(out=b1t[:], in_=b1r[:])

            # ALL loads ride ONE ring (sync) in exact consumption order: the
            # queue is FIFO and stripes across all 16 SDMA engines, so strict
            # ordering gives perfect pacing (the W2 stream can never starve
            # the W1 stream and vice versa).  Every tile has its own buffer:
            # no WAR throttling, the ring order alone paces the stream.
            w1ts: list = [None] * len(GRPS)
            w2ts: list = [None] * NP

            def load_w1(g, f0, gw):
                t = w1p.tile(
                    [128, KD * gw * 128], dt_io, tag=f"w1_{g}", name=f"w1g{g}"
                )
                nc.sync.dma_start(
                    out=t[:], in_=w1[:, KD * 128 * f0 : KD * 128 * (f0 + gw)]
                )
                w1ts[g] = t

            def load_w2(p):
                t = w2p.tile([128, W2P * D], dt_io, tag=f"w2_{p}", name=f"w2t{p}")
                nc.sync.dma_start(out=t[:], in_=w2[p])
                w2ts[p] = t

            xws = []
            for s in range(n_slabs):
                xw = xp.tile([128, KD * CS], dt_io, tag=f"xw{s}", name=f"xw{s}")
                if s == 0:
                    nc.sync.dma_start(out=xw[:], in_=xX[s])
                xws.append(xw)
            goffs = []
            f0 = 0
            for g, gw in enumerate(GRPS):
                goffs.append((f0, gw))
                f0 += gw
            # interleave: x, g0, g1, w2_0, g2, w2_1, g3, ..., trailing w2
            m = 0
            for g, gw in enumerate(GRPS):
                if g >= 2 and m < NP:
                    load_w2(m)
                    m += 1
                load_w1(g, *goffs[g])
            while m < NP:
                load_w2(m)
                m += 1

            for s in range(n_slabs):
                xw = xws[s]
                if s > 0:
                    nc.sync.dma_start(out=xw[:], in_=xX[s])
                py = [
                    ps_y.tile([128, CS], f32, tag=f"y{dd}", name=f"py{dd}")
                    for dd in range(KD)
                ]

                def emit_mm2(f, ht):
                    # yT += W2[f-chunk, :]^T @ hT[f-chunk]
                    p, r = divmod(f, W2P)
                    for dd in range(KD):
                        nc.tensor.matmul(
                            py[dd][:],
                            w2ts[p][:, r * D + dd * 128 : r * D + (dd + 1) * 128],
                            ht[:],
                            start=(f == 0),
                            stop=(f == KF - 1),
                        )

                pend = None  # (f, ht) whose mm2 is deferred one chunk
                for g, (f0, gw) in enumerate(goffs):
                    w1g = w1ts[g]
                    for j in range(gw):
                        f = f0 + j
                        # hT[f-chunk] = silu(sum_d W1[d, f-chunk]^T @ xT[d] + b1)
                        ph = ps_h.tile([128, CS], f32, tag="hps", name="ph")
                        for d in range(KD):
                            nc.tensor.matmul(
                                ph[:],
                                w1g[:, (d * gw + j) * 128 : (d * gw + j + 1) * 128],
                                xw[:, d * CS : (d + 1) * CS],
                                start=(d == 0),
                                stop=(d == KD - 1),
                            )
                        ht = hp.tile([128, CS], dt_io, tag="ht", name="ht")
                        nc.scalar.activation(ht[:], ph[:], silu, bias=b1t[:, f : f + 1])
                        # mm2 for the PREVIOUS chunk: its silu ran while this
                        # chunk's mm1 was on the PE, so the PE never waits on
                        # the ACT engine
                        if pend is not None:
                            emit_mm2(*pend)
                        pend = (f, ht)
                if pend is not None:
                    emit_mm2(*pend)
                    pend = None

                # tail: evacuate PSUM on both DVE (dd<3) and ACT (dd>=3),
                # casting to dt_io; each evac engine then triggers its own
                # half's store (vector directly, scalar's half via the idle
                # gpsimd ring) so no load traffic sits in front of it
                yt = yp.tile([128, KD * CS], dt_io, tag="yt", name="yt")
                half = KD // 2
                for dd in range(KD):
                    if dd < half:
                        nc.vector.tensor_copy(
                            yt[:, dd * CS : (dd + 1) * CS], py[dd][:]
                        )
                    else:
                        nc.scalar.copy(yt[:, dd * CS : (dd + 1) * CS], py[dd][:])
                nc.gpsimd.dma_start(
                    out=yX[s][:, 0 : half * CS], in_=yt[:, 0 : half * CS]
                )
                nc.scalar.dma_start(
                    out=yX[s][:, half * CS :], in_=yt[:, half * CS :]
                )

    nc.compile()
    return nc


def _get_bass(C: int, n_slabs: int, mode: str, D: int, F: int):
    key = (C, n_slabs, mode, D, F, FG, W2P, N_WARM)
    if key not in _CACHE:
        _CACHE[key] = _build_bass(C, n_slabs, mode, D, F)
    return _CACHE[key]


def _gate_host(x: np.ndarray, Wg: np.ndarray):
    """Top-1 gating in float64: returns (expert_idx [T], gate [T] f32)."""
    logits = x.astype(np.float64) @ Wg.astype(np.float64)
    m = logits.max(-1, keepdims=True)
    p = np.exp(logits - m)
    p /= p.sum(-1, keepdims=True)
    return p.argmax(-1), p.max(-1).astype(np.float32)


def _kernel_numpy(x, Wg, W1, b1, W2, b2):
    """Reference-equivalent fallback (host only)."""
    idx, gate = _gate_host(x, Wg)
    out = np.zeros_like(x)
    for e in range(W1.shape[0]):
        ids = np.nonzero(idx == e)[0]
        if ids.size == 0:
            continue
        h = x[ids] @ W1[e] + b1[e]
        h = h * (1.0 / (1.0 + np.exp(-h)))
        out[ids] = gate[ids, None] * (h @ W2[e] + b2[e])
    return out


def _pack_weights(W1, b1, W2, np_io):
    """Per-expert weight images (cached across calls: weights don't change)."""
    key = (W1.ctypes.data, W2.ctypes.data, W1.shape, W2.shape, np_io)
    if key in _PACK_CACHE:
        return _PACK_CACHE[key]
    E, D, F = W1.shape
    KD, KF = D // 128, F // 128
    NP = KF // W2P
    grps = _w1_groups(KF)
    packed = []
    for e in range(E):
        w1e = W1[e].reshape(KD, 128, KF, 128)
        parts = []
        f0 = 0
        for gw in grps:
            blk = w1e[:, :, f0 : f0 + gw]  # [KD, 128, gw, 128]
            parts.append(blk.transpose(1, 0, 2, 3).reshape(128, KD * gw * 128))
            f0 += gw
        w1r = np.concatenate(parts, axis=1)  # [128, KD*F]
        w2r = (
            W2[e]
            .reshape(NP, W2P, 128, D)
            .transpose(0, 2, 1, 3)
            .reshape(NP, 128, W2P * D)
        )
        packed.append(
            {
                "w1": np.ascontiguousarray(w1r).astype(np_io, copy=False),
                "w2": np.ascontiguousarray(w2r).astype(np_io, copy=False),
                "b1r": np.ascontiguousarray(b1[e].reshape(KF, 128).T),
            }
        )
    _PACK_CACHE[key] = packed
    return packed


def kernel(hidden_states, Wg, W1, b1, W2, b2):
    hidden_states = np.asarray(hidden_states)
    Wg = np.asarray(Wg, dtype=np.float32)
    W1 = np.asarray(W1, dtype=np.float32)
    b1 = np.asarray(b1, dtype=np.float32)
    W2 = np.asarray(W2, dtype=np.float32)
    b2 = np.asarray(b2, dtype=np.float32)

    orig_shape = hidden_states.shape
    D = orig_shape[-1]
    x = np.ascontiguousarray(hidden_states, dtype=np.float32).reshape(-1, D)
    E, _, F = W1.shape
    KD, KF = D // 128, F // 128

    if E != N_CORES or D % 128 != 0 or F % 128 != 0 or KF % W2P != 0:
        return _kernel_numpy(x, Wg, W1, b1, W2, b2).reshape(orig_shape)

    idx, gate = _gate_host(x, Wg)
    order = np.argsort(idx, kind="stable")
    counts = np.bincount(idx, minlength=E)
    starts = np.concatenate([[0], np.cumsum(counts)])

    # Capacity: common padded token count per core. Slab width must be
    # 256..512 (PSUM bank limit / fp32r fast path).
    C = max(256, _roundup(int(counts.max()), 16))
    n_slabs = -(-C // 512)
    C = n_slabs * max(256, _roundup(-(-C // n_slabs), 16))
    CS = C // n_slabs

    mode = MODE
    np_io = np.float32
    if mode == "bf16":
        import ml_dtypes

        np_io = ml_dtypes.bfloat16

    nc = _get_bass(C, n_slabs, mode, D, F)

    wpacked = _pack_weights(W1, b1, W2, np_io)
    in_maps = []
    for e in range(E):
        ids = order[starts[e] : starts[e + 1]]
        xe = np.zeros((C, D), dtype=np.float32)
        xe[: ids.size] = x[ids]
        # [n_slabs, 128, KD*CS] SBUF image (contiguous per partition row)
        xr = xe.reshape(n_slabs, CS, KD, 128).transpose(0, 3, 2, 1)
        xr = np.ascontiguousarray(xr.reshape(n_slabs, 128, KD * CS))
        in_maps.append({"xX": xr.astype(np_io, copy=False), **wpacked[e]})

    res = run_bass_kernel_spmd(nc, in_maps, list(range(N_CORES)))

    out = np.zeros_like(x)
    for e in range(E):
        ids = order[starts[e] : starts[e + 1]]
        if ids.size == 0:
            continue
        yr = np.asarray(res.results[e]["yX"], dtype=np.float32)
        y = yr.reshape(n_slabs, 128, KD, CS).transpose(0, 3, 2, 1).reshape(C, D)
        out[ids] = gate[ids, None] * (y[: ids.size] + b2[e])
    return out.reshape(orig_shape)


# revision 8
# speedup vs baseline: 1.0992x; 1.0056x over previous
"""MoE top-1 feed-forward (DeepSpeed-style) on 8 Trainium2 NeuronCores.

Strategy (expert parallelism, per the sharding hint):
  - Host computes the (tiny) gate: logits = x @ Wg, softmax, top-1 expert id
    and gate prob per token (float64 for a faithful argmax).
  - Tokens are dispatched to the core owning their expert (core e holds
    W1[e]/b1[e]/W2[e]/b2[e]); each core's token batch is padded to a common
    capacity C so all 8 cores run one SPMD program.
  - Each core runs the dense FFN for its tokens:
        hT = silu(W1^T @ xT + b1);  yT = W2^T @ hT
    with tokens laid out along the free (moving) dimension so no transposes
    are needed on device: xT is [D, C], hT is [F, C], yT is [D, C].
  - All images are host-packed to the EXACT SBUF layout so every DMA is
    contiguous per partition row (few, large descriptors; the descriptor
    storm of a strided gather was the dominant head/tail latency).
  - bf16 weights/activations (f32 PSUM accumulate): halves HBM traffic vs
    f32 and runs the PE at full rate; the problem sits at the roofline
    ridge (~31us HBM, ~33us PE per core).
  - Rings: sync=x+W1(+y), gpsimd=W2+b1(+y), scalar=silu+evac, vector=
    fanout+evac, tensor=matmul only.  A short junk-matmul warmup ramps the
    PE p-state (0.65->2.4GHz) while the first loads land.
  - Host combines: out[token] = gate * (y + b2[expert]).
"""

import os
import sys

import numpy as np

try:
    import concourse.mybir as mybir  # noqa: F401
except ModuleNotFoundError:  # fallback if the site hooks aren't installed
    sys.path.insert(0, "/opt/trn_rl_repo")

import concourse.mybir as mybir
import concourse.tile as tile
from concourse import bacc
from concourse.bass_utils import run_bass_kernel_spmd

N_CORES = 8

# Compute dtype for the matmuls:
#   "bf16" - weights/activations cast to bfloat16 (f32 PSUM accumulate)
#   "f32r" - fp32 data, PE's replicated-fp32 mode (full rate at N>=256)
#   "f32"  - plain fp32 matmuls (4x slower PE)
MODE = os.environ.get("BASS_MOE_MODE", "bf16")

FG = int(os.environ.get("BASS_MOE_FG", "2"))  # steady-state f-chunks per W1 group
W2P = int(os.environ.get("BASS_MOE_W2P", "2"))  # f-chunks per W2 pair-tile
N_WARM = int(os.environ.get("BASS_MOE_WARM", "6"))  # PE p-state warmup matmuls


def _w1_groups(KF):
    """F-chunk widths per W1 group: small leading groups let the PE start
    before a whole FG-wide image lands."""
    lead = [1, 1] if FG > 1 and KF > 8 else []
    rem = KF - sum(lead)
    groups = list(lead)
    while rem > 0:
        w = min(FG, rem)
        groups.append(w)
        rem -= w
    return groups


_CACHE: dict = {}
_PACK_CACHE: dict = {}


def _roundup(a: int, m: int) -> int:
    return -(-a // m) * m


def _build_bass(C: int, n_slabs: int, mode: str, D: int, F: int):
    """Build + compile the per-core Bass program for capacity C (divisible by
    n_slabs; slab width CS = C/n_slabs must be 256..512)."""
    f32 = mybir.dt.float32
    if mode == "bf16":
        dt_io = mybir.dt.bfloat16
    elif mode == "f32r":
        dt_io = mybir.dt.float32r
    else:
        dt_io = f32

    KD, KF = D // 128, F // 128
    GRPS = _w1_groups(KF)
    NP = KF // W2P  # number of W2 pair-tiles
    CS = C // n_slabs
    assert C % n_slabs == 0 and 256 <= CS <= 512

    nc = bacc.Bacc(None, target_bir_lowering=False, debug=False)
    # Host-packed images (see kernel() for the packing); every load/store is
    # contiguous per partition row:
    #   xX   [n_slabs, 128, KD*CS]  xX[s,p,d*CS+c] = x[s*CS+c, d*128+p]
    #   w1   [128, KD*F]            flat group images; group g at column
    #                               offset KD*128*sum(GRPS[:g]), blocks (d, j)
    #                               within a group at (d*gw+j)*128
    #   w2   [NP, 128, W2P*D]       w2[p] f-chunk r=f-p*W2P at cols r*D
    #   b1r  [128, KF]              b1[f*128+p] at [p, f]
    #   yX   [n_slabs, 128, KD*CS]  output, same layout as xX (dt_io)
    xX = nc.dram_tensor("xX", [n_slabs, 128, KD * CS], dt_io, kind="ExternalInput")
    w1 = nc.dram_tensor("w1", [128, KD * F], dt_io, kind="ExternalInput")
    w2 = nc.dram_tensor("w2", [NP, 128, W2P * D], dt_io, kind="ExternalInput")
    b1r = nc.dram_tensor("b1r", [128, KF], f32, kind="ExternalInput")
    yX = nc.dram_tensor("yX", [n_slabs, 128, KD * CS], dt_io, kind="ExternalOutput")

    silu = mybir.ActivationFunctionType.Silu

    with tile.TileContext(nc) as tc:
        with (
            tc.tile_pool(name="xp", bufs=2) as xp,
            tc.tile_pool(name="w1p", bufs=1) as w1p,
            tc.tile_pool(name="w2p", bufs=1) as w2p,
            tc.tile_pool(name="hp", bufs=4) as hp,
            tc.tile_pool(name="bp", bufs=1) as bp,
            tc.tile_pool(name="yp", bufs=2) as yp,
            tc.tile_pool(name="jp", bufs=1) as jp,
            tc.tile_pool(name="ps_h", bufs=2, space="PSUM") as ps_h,
            tc.tile_pool(name="ps_y", bufs=1, space="PSUM") as ps_y,
        ):
            # PE p-state warmup: junk matmuls with no data deps keep the PE
            # busy from the end of the preamble so the 0.65/1.2GHz clock ramp
            # is spent while the first weight/activation DMAs land, not on
            # real work.  gpsimd exits the preamble earliest and is idle.
            junk = jp.tile([128, CS], dt_io, tag="junk", name="junk")
            nc.gpsimd.memset(junk[:], 0)
            for i in range(N_WARM):
                pj = ps_h.tile([128, CS], f32, tag="hps", name="pjunk")
                nc.tensor.matmul(
                    pj[:], junk[:, 0:128], junk[:], start=True, stop=True
                )

            b1t = bp.tile([128, KF], f32, tag="b1", name="b1t")
            nc.gpsimd.dma_start(out=b1t[:], in_=b1r[:])

            # Loads ride the sync ring in exact consumption order: the queue
            # is FIFO and stripes across all 16 SDMA engines, so strict
            # ordering gives perfect pacing (the W2 stream can never starve
            # the W1 stream and vice versa).  Every tile has its own buffer:
            # no WAR throttling, the ring order alone paces the stream.  The
            # x image goes on the scalar ring (ACT exits the preamble ~0.6us
            # before SP), so x and w1g0 land concurrently right after the
            # preamble.
            w1ts: list = [None] * len(GRPS)
            w2ts: list = [None] * NP

            def load_w1(g, f0, gw):
                t = w1p.tile(
                    [128, KD * gw * 128], dt_io, tag=f"w1_{g}", name=f"w1g{g}"
                )
                nc.sync.dma_start(
                    out=t[:], in_=w1[:, KD * 128 * f0 : KD * 128 * (f0 + gw)]
                )
                w1ts[g] = t

            def load_w2(p):
                t = w2p.tile([128, W2P * D], dt_io, tag=f"w2_{p}", name=f"w2t{p}")
                nc.sync.dma_start(out=t[:], in_=w2[p])
                w2ts[p] = t

            xws = []
            for s in range(n_slabs):
                xw = xp.tile([128, KD * CS], dt_io, tag=f"xw{s}", name=f"xw{s}")
                if s == 0:
                    nc.scalar.dma_start(out=xw[:], in_=xX[s])
                xws.append(xw)
            goffs = []
            f0 = 0
            for g, gw in enumerate(GRPS):
                goffs.append((f0, gw))
                f0 += gw
            # interleave: g0, g1, w2_0, g2, w2_1, g3, ..., trailing w2
            m = 0
            for g, gw in enumerate(GRPS):
                if g >= 2 and m < NP:
                    load_w2(m)
                    m += 1
                load_w1(g, *goffs[g])
            while m < NP:
                load_w2(m)
                m += 1

            for s in range(n_slabs):
                xw = xws[s]
                if s > 0:
                    nc.sync.dma_start(out=xw[:], in_=xX[s])
                py = [
                    ps_y.tile([128, CS], f32, tag=f"y{dd}", name=f"py{dd}")
                    for dd in range(KD)
                ]

                def emit_mm2(f, ht):
                    # yT += W2[f-chunk, :]^T @ hT[f-chunk]
                    p, r = divmod(f, W2P)
                    for dd in range(KD):
                        nc.tensor.matmul(
                            py[dd][:],
                            w2ts[p][:, r * D + dd * 128 : r * D + (dd + 1) * 128],
                            ht[:],
                            start=(f == 0),
                            stop=(f == KF - 1),
                        )

                pend = None  # (f, ht) whose mm2 is deferred one chunk
                for g, (f0, gw) in enumerate(goffs):
                    w1g = w1ts[g]
                    for j in range(gw):
                        f = f0 + j
                        # hT[f-chunk] = silu(sum_d W1[d, f-chunk]^T @ xT[d] + b1)
                        ph = ps_h.tile([128, CS], f32, tag="hps", name="ph")
                        for d in range(KD):
                            nc.tensor.matmul(
                                ph[:],
                                w1g[:, (d * gw + j) * 128 : (d * gw + j + 1) * 128],
                                xw[:, d * CS : (d + 1) * CS],
                                start=(d == 0),
                                stop=(d == KD - 1),
                            )
                        ht = hp.tile([128, CS], dt_io, tag="ht", name="ht")
                        nc.scalar.activation(ht[:], ph[:], silu, bias=b1t[:, f : f + 1])
                        # mm2 for the PREVIOUS chunk: its silu ran while this
                        # chunk's mm1 was on the PE, so the PE never waits on
                        # the ACT engine
                        if pend is not None:
                            emit_mm2(*pend)
                        pend = (f, ht)
                if pend is not None:
                    emit_mm2(*pend)
                    pend = None

                # tail: evacuate PSUM on both DVE (dd<3) and ACT (dd>=3),
                # casting to dt_io; each evac engine then triggers its own
                # half's store (vector directly, scalar's half via the idle
                # gpsimd ring) so no load traffic sits in front of it
                yt = yp.tile([128, KD * CS], dt_io, tag="yt", name="yt")
                half = KD // 2
                for dd in range(KD):
                    if dd < half:
                        nc.vector.tensor_copy(
                            yt[:, dd * CS : (dd + 1) * CS], py[dd][:]
                        )
                    else:
                        nc.scalar.copy(yt[:, dd * CS : (dd + 1) * CS], py[dd][:])
                nc.gpsimd.dma_start(
                    out=yX[s][:, 0 : half * CS], in_=yt[:, 0 : half * CS]
                )
                nc.scalar.dma_start(
                    out=yX[s][:, half * CS :], in_=yt[:, half * CS :]
                )

    nc.compile()
    return nc


def _get_bass(C: int, n_slabs: int, mode: str, D: int, F: int):
    key = (C, n_slabs, mode, D, F, FG, W2P, N_WARM)
    if key not in _CACHE:
        _CACHE[key] = _build_bass(C, n_slabs, mode, D, F)
    return _CACHE[key]


def _gate_host(x: np.ndarray, Wg: np.ndarray):
    """Top-1 gating in float64: returns (expert_idx [T], gate [T] f32)."""
    logits = x.astype(np.float64) @ Wg.astype(np.float64)
    m = logits.max(-1, keepdims=True)
    p = np.exp(logits - m)
    p /= p.sum(-1, keepdims=True)
    return p.argmax(-1), p.max(-1).astype(np.float32)


def _kernel_numpy(x, Wg, W1, b1, W2, b2):
    """Reference-equivalent fallback (host only)."""
    idx, gate = _gate_host(x, Wg)
    out = np.zeros_like(x)
    for e in range(W1.shape[0]):
        ids = np.nonzero(idx == e)[0]
        if ids.size == 0:
            continue
        h = x[ids] @ W1[e] + b1[e]
        h = h * (1.0 / (1.0 + np.exp(-h)))
        out[ids] = gate[ids, None] * (h @ W2[e] + b2[e])
    return out


def _pack_weights(W1, b1, W2, np_io):
    """Per-expert weight images (cached across calls: weights don't change)."""
    key = (W1.ctypes.data, W2.ctypes.data, W1.shape, W2.shape, np_io)
    if key in _PACK_CACHE:
        return _PACK_CACHE[key]
    E, D, F = W1.shape
    KD, KF = D // 128, F // 128
    NP = KF // W2P
    grps = _w1_groups(KF)
    packed = []
    for e in range(E):
        w1e = W1[e].reshape(KD, 128, KF, 128)
        parts = []
        f0 = 0
        for gw in grps:
            blk = w1e[:, :, f0 : f0 + gw]  # [KD, 128, gw, 128]
            parts.append(blk.transpose(1, 0, 2, 3).reshape(128, KD * gw * 128))
            f0 += gw
        w1r = np.concatenate(parts, axis=1)  # [128, KD*F]
        w2r = (
            W2[e]
            .reshape(NP, W2P, 128, D)
            .transpose(0, 2, 1, 3)
            .reshape(NP, 128, W2P * D)
        )
        packed.append(
            {
                "w1": np.ascontiguousarray(w1r).astype(np_io, copy=False),
                "w2": np.ascontiguousarray(w2r).astype(np_io, copy=False),
                "b1r": np.ascontiguousarray(b1[e].reshape(KF, 128).T),
            }
        )
    _PACK_CACHE[key] = packed
    return packed


def kernel(hidden_states, Wg, W1, b1, W2, b2):
    hidden_states = np.asarray(hidden_states)
    Wg = np.asarray(Wg, dtype=np.float32)
    W1 = np.asarray(W1, dtype=np.float32)
    b1 = np.asarray(b1, dtype=np.float32)
    W2 = np.asarray(W2, dtype=np.float32)
    b2 = np.asarray(b2, dtype=np.float32)

    orig_shape = hidden_states.shape
    D = orig_shape[-1]
    x = np.ascontiguousarray(hidden_states, dtype=np.float32).reshape(-1, D)
    E, _, F = W1.shape
    KD, KF = D // 128, F // 128

    if E != N_CORES or D % 128 != 0 or F % 128 != 0 or KF % W2P != 0:
        return _kernel_numpy(x, Wg, W1, b1, W2, b2).reshape(orig_shape)

    idx, gate = _gate_host(x, Wg)
    order = np.argsort(idx, kind="stable")
    counts = np.bincount(idx, minlength=E)
    starts = np.concatenate([[0], np.cumsum(counts)])

    # Capacity: common padded token count per core. Slab width must be
    # 256..512 (PSUM bank limit / fp32r fast path).
    C = max(256, _roundup(int(counts.max()), 16))
    n_slabs = -(-C // 512)
    C = n_slabs * max(256, _roundup(-(-C // n_slabs), 16))
    CS = C // n_slabs

    mode = MODE
    np_io = np.float32
    if mode == "bf16":
        import ml_dtypes

        np_io = ml_dtypes.bfloat16

    nc = _get_bass(C, n_slabs, mode, D, F)

    wpacked = _pack_weights(W1, b1, W2, np_io)
    in_maps = []
    for e in range(E):
        ids = order[starts[e] : starts[e + 1]]
        xe = np.zeros((C, D), dtype=np.float32)
        xe[: ids.size] = x[ids]
        # [n_slabs, 128, KD*CS] SBUF image (contiguous per partition row)
        xr = xe.reshape(n_slabs, CS, KD, 128).transpose(0, 3, 2, 1)
        xr = np.ascontiguousarray(xr.reshape(n_slabs, 128, KD * CS))
        in_maps.append({"xX": xr.astype(np_io, copy=False), **wpacked[e]})

    res = run_bass_kernel_spmd(nc, in_maps, list(range(N_CORES)))

    out = np.zeros_like(x)
    for e in range(E):
        ids = order[starts[e] : starts[e + 1]]
        if ids.size == 0:
            continue
        yr = np.asarray(res.results[e]["yX"], dtype=np.float32)
        y = yr.reshape(n_slabs, 128, KD, CS).transpose(0, 3, 2, 1).reshape(C, D)
        out[ids] = gate[ids, None] * (y[: ids.size] + b2[e])
    return out.reshape(orig_shape)


# revision 11
# speedup vs baseline: 1.1126x; 1.0122x over previous
"""MoE top-1 feed-forward (DeepSpeed-style) on 8 Trainium2 NeuronCores.

Strategy (expert parallelism, per the sharding hint):
  - Host computes the (tiny) gate: logits = x @ Wg, softmax, top-1 expert id
    and gate prob per token (float64 for a faithful argmax).
  - Tokens are dispatched to the core owning their expert (core e holds
    W1[e]/b1[e]/W2[e]/b2[e]); each core's token batch is padded to a common
    capacity C so all 8 cores run one SPMD program.
  - Each core runs the dense FFN for its tokens:
        hT = silu(W1^T @ xT + b1);  yT = W2^T @ hT
    with tokens laid out along the free (moving) dimension so no transposes
    are needed on device: xT is [D, C], hT is [F, C], yT is [D, C].
  - All images are host-packed to the EXACT SBUF layout so every DMA is
    contiguous per partition row (few, large descriptors; the descriptor
    storm of a strided gather was the dominant head/tail latency).
  - bf16 weights/activations (f32 PSUM accumulate): halves HBM traffic vs
    f32 and runs the PE at full rate; the problem sits at the roofline
    ridge (~31us HBM, ~33us PE per core).
  - Rings: sync=x+W1(+y), gpsimd=W2+b1(+y), scalar=silu+evac, vector=
    fanout+evac, tensor=matmul only.  A short junk-matmul warmup ramps the
    PE p-state (0.65->2.4GHz) while the first loads land.
  - Host combines: out[token] = gate * (y + b2[expert]).
"""

import os
import sys

import numpy as np

try:
    import concourse.mybir as mybir  # noqa: F401
except ModuleNotFoundError:  # fallback if the site hooks aren't installed
    sys.path.insert(0, "/opt/trn_rl_repo")

import concourse.mybir as mybir
import concourse.tile as tile
from concourse import bacc
from concourse.bass_utils import run_bass_kernel_spmd

N_CORES = 8

# Compute dtype for the matmuls:
#   "bf16" - weights/activations cast to bfloat16 (f32 PSUM accumulate)
#   "f32r" - fp32 data, PE's replicated-fp32 mode (full rate at N>=256)
#   "f32"  - plain fp32 matmuls (4x slower PE)
MODE = os.environ.get("BASS_MOE_MODE", "bf16")

FG = int(os.environ.get("BASS_MOE_FG", "2"))  # steady-state f-chunks per W1 group
W2P = int(os.environ.get("BASS_MOE_W2P", "2"))  # f-chunks per W2 pair-tile
N_WARM = int(os.environ.get("BASS_MOE_WARM", "12"))  # PE p-state warmup matmuls


def _w1_groups(KF):
    """F-chunk widths per W1 group: small leading groups let the PE start
    before a whole FG-wide image lands."""
    lead = [1, 1] if FG > 1 and KF > 8 else []
    rem = KF - sum(lead)
    groups = list(lead)
    while rem > 0:
        w = min(FG, rem)
        groups.append(w)
        rem -= w
    return groups


_CACHE: dict = {}
_PACK_CACHE: dict = {}


def _roundup(a: int, m: int) -> int:
    return -(-a // m) * m


def _build_bass(C: int, n_slabs: int, mode: str, D: int, F: int):
    """Build + compile the per-core Bass program for capacity C (divisible by
    n_slabs; slab width CS = C/n_slabs must be 256..512)."""
    f32 = mybir.dt.float32
    if mode == "bf16":
        dt_io = mybir.dt.bfloat16
    elif mode == "f32r":
        dt_io = mybir.dt.float32r
    else:
        dt_io = f32

    KD, KF = D // 128, F // 128
    GRPS = _w1_groups(KF)
    NP = KF // W2P  # number of W2 pair-tiles
    CS = C // n_slabs
    assert C % n_slabs == 0 and 256 <= CS <= 512

    nc = bacc.Bacc(None, target_bir_lowering=False, debug=False)
    # Host-packed images (see kernel() for the packing); every load/store is
    # contiguous per partition row:
    #   xX   [n_slabs, 128, KD*CS]  xX[s,p,d*CS+c] = x[s*CS+c, d*128+p]
    #   w1   [128, KD*F]            flat group images; group g at column
    #                               offset KD*128*sum(GRPS[:g]), blocks (d, j)
    #                               within a group at (d*gw+j)*128
    #   w2   [NP, 128, W2P*D]       w2[p] f-chunk r=f-p*W2P at cols r*D
    #   b1r  [128, KF]              b1[f*128+p] at [p, f]
    #   yX   [n_slabs, 128, KD*CS]  output, same layout as xX (dt_io)
    xX = nc.dram_tensor("xX", [n_slabs, 128, KD * CS], dt_io, kind="ExternalInput")
    w1 = nc.dram_tensor("w1", [128, KD * F], dt_io, kind="ExternalInput")
    w2 = nc.dram_tensor("w2", [NP, 128, W2P * D], dt_io, kind="ExternalInput")
    b1r = nc.dram_tensor("b1r", [128, KF], f32, kind="ExternalInput")
    yX = nc.dram_tensor("yX", [n_slabs, 128, KD * CS], dt_io, kind="ExternalOutput")

    silu = mybir.ActivationFunctionType.Silu

    with tile.TileContext(nc) as tc:
        with (
            tc.tile_pool(name="xp", bufs=2) as xp,
            tc.tile_pool(name="w1p", bufs=1) as w1p,
            tc.tile_pool(name="w2p", bufs=1) as w2p,
            tc.tile_pool(name="hp", bufs=4) as hp,
            tc.tile_pool(name="bp", bufs=1) as bp,
            tc.tile_pool(name="yp", bufs=2) as yp,
            tc.tile_pool(name="jp", bufs=1) as jp,
            tc.tile_pool(name="ps_h", bufs=2, space="PSUM") as ps_h,
            tc.tile_pool(name="ps_y", bufs=1, space="PSUM") as ps_y,
        ):
            # PE p-state warmup: junk matmuls with no data deps keep the PE
            # busy from the end of the preamble so the 0.65/1.2GHz clock ramp
            # is spent while the first weight/activation DMAs land, not on
            # real work.  gpsimd exits the preamble earliest and is idle.
            junk = jp.tile([128, CS], dt_io, tag="junk", name="junk")
            nc.gpsimd.memset(junk[:], 0)
            for i in range(N_WARM):
                pj = ps_h.tile([128, CS], f32, tag="hps", name="pjunk")
                nc.tensor.matmul(
                    pj[:], junk[:, 0:128], junk[:], start=True, stop=True
                )

            b1t = bp.tile([128, KF], f32, tag="b1", name="b1t")
            nc.gpsimd.dma_start(out=b1t[:], in_=b1r[:])

            # Loads ride the sync ring in exact consumption order: the queue
            # is FIFO and stripes across all 16 SDMA engines, so strict
            # ordering gives perfect pacing (the W2 stream can never starve
            # the W1 stream and vice versa).  Every tile has its own buffer:
            # no WAR throttling, the ring order alone paces the stream.  The
            # x image goes on the scalar ring (ACT exits the preamble ~0.6us
            # before SP), so x and w1g0 land concurrently right after the
            # preamble.
            w1ts: list = [None] * len(GRPS)
            w2ts: list = [None] * NP

            def load_w1(g, f0, gw):
                t = w1p.tile(
                    [128, KD * gw * 128], dt_io, tag=f"w1_{g}", name=f"w1g{g}"
                )
                nc.sync.dma_start(
                    out=t[:], in_=w1[:, KD * 128 * f0 : KD * 128 * (f0 + gw)]
                )
                w1ts[g] = t

            def load_w2(p):
                t = w2p.tile([128, W2P * D], dt_io, tag=f"w2_{p}", name=f"w2t{p}")
                nc.sync.dma_start(out=t[:], in_=w2[p])
                w2ts[p] = t

            xws = []
            for s in range(n_slabs):
                xw = xp.tile([128, KD * CS], dt_io, tag=f"xw{s}", name=f"xw{s}")
                if s == 0:
                    nc.sync.dma_start(out=xw[:], in_=xX[s])
                xws.append(xw)
            goffs = []
            f0 = 0
            for g, gw in enumerate(GRPS):
                goffs.append((f0, gw))
                f0 += gw
            # interleave: g0, g1, w2_0, g2, w2_1, g3, ..., trailing w2
            m = 0
            for g, gw in enumerate(GRPS):
                if g >= 2 and m < NP:
                    load_w2(m)
                    m += 1
                load_w1(g, *goffs[g])
            while m < NP:
                load_w2(m)
                m += 1

            for s in range(n_slabs):
                xw = xws[s]
                if s > 0:
                    nc.sync.dma_start(out=xw[:], in_=xX[s])
                py = [
                    ps_y.tile([128, CS], f32, tag=f"y{dd}", name=f"py{dd}")
                    for dd in range(KD)
                ]

                def emit_mm2(f, ht):
                    # yT += W2[f-chunk, :]^T @ hT[f-chunk]
                    p, r = divmod(f, W2P)
                    for dd in range(KD):
                        nc.tensor.matmul(
                            py[dd][:],
                            w2ts[p][:, r * D + dd * 128 : r * D + (dd + 1) * 128],
                            ht[:],
                            start=(f == 0),
                            stop=(f == KF - 1),
                        )

                pend = None  # (f, ht) whose mm2 is deferred one chunk
                for g, (f0, gw) in enumerate(goffs):
                    w1g = w1ts[g]
                    for j in range(gw):
                        f = f0 + j
                        # hT[f-chunk] = silu(sum_d W1[d, f-chunk]^T @ xT[d] + b1)
                        ph = ps_h.tile([128, CS], f32, tag="hps", name="ph")
                        for d in range(KD):
                            nc.tensor.matmul(
                                ph[:],
                                w1g[:, (d * gw + j) * 128 : (d * gw + j + 1) * 128],
                                xw[:, d * CS : (d + 1) * CS],
                                start=(d == 0),
                                stop=(d == KD - 1),
                            )
                        ht = hp.tile([128, CS], dt_io, tag="ht", name="ht")
                        nc.scalar.activation(ht[:], ph[:], silu, bias=b1t[:, f : f + 1])
                        # mm2 for the PREVIOUS chunk: its silu ran while this
                        # chunk's mm1 was on the PE, so the PE never waits on
                        # the ACT engine
                        if pend is not None:
                            emit_mm2(*pend)
                        pend = (f, ht)
                if pend is not None:
                    emit_mm2(*pend)
                    pend = None

                # tail: evacuate PSUM on both DVE (dd<3) and ACT (dd>=3),
                # casting to dt_io; stream out both halves on the sync ring
                # (idle and warm after the load stream) as soon as each
                # half's evacs land
                yt = yp.tile([128, KD * CS], dt_io, tag="yt", name="yt")
                half = KD // 2
                for dd in range(KD):
                    if dd < half:
                        nc.vector.tensor_copy(
                            yt[:, dd * CS : (dd + 1) * CS], py[dd][:]
                        )
                    else:
                        nc.scalar.copy(yt[:, dd * CS : (dd + 1) * CS], py[dd][:])
                nc.sync.dma_start(
                    out=yX[s][:, 0 : half * CS], in_=yt[:, 0 : half * CS]
                )
                nc.sync.dma_start(
                    out=yX[s][:, half * CS :], in_=yt[:, half * CS :]
                )

    nc.compile()
    return nc


def _get_bass(C: int, n_slabs: int, mode: str, D: int, F: int):
    key = (C, n_slabs, mode, D, F, FG, W2P, N_WARM)
    if key not in _CACHE:
        _CACHE[key] = _build_bass(C, n_slabs, mode, D, F)
    return _CACHE[key]


def _gate_host(x: np.ndarray, Wg: np.ndarray):
    """Top-1 gating in float64: returns (expert_idx [T], gate [T] f32)."""
    logits = x.astype(np.float64) @ Wg.astype(np.float64)
    m = logits.max(-1, keepdims=True)
    p = np.exp(logits - m)
    p /= p.sum(-1, keepdims=True)
    return p.argmax(-1), p.max(-1).astype(np.float32)


def _kernel_numpy(x, Wg, W1, b1, W2, b2):
    """Reference-equivalent fallback (host only)."""
    idx, gate = _gate_host(x, Wg)
    out = np.zeros_like(x)
    for e in range(W1.shape[0]):
        ids = np.nonzero(idx == e)[0]
        if ids.size == 0:
            continue
        h = x[ids] @ W1[e] + b1[e]
        h = h * (1.0 / (1.0 + np.exp(-h)))
        out[ids] = gate[ids, None] * (h @ W2[e] + b2[e])
    return out


def _pack_weights(W1, b1, W2, np_io):
    """Per-expert weight images (cached across calls: weights don't change)."""
    key = (W1.ctypes.data, W2.ctypes.data, W1.shape, W2.shape, np_io)
    if key in _PACK_CACHE:
        return _PACK_CACHE[key]
    E, D, F = W1.shape
    KD, KF = D // 128, F // 128
    NP = KF // W2P
    grps = _w1_groups(KF)
    packed = []
    for e in range(E):
        w1e = W1[e].reshape(KD, 128, KF, 128)
        parts = []
        f0 = 0
        for gw in grps:
            blk = w1e[:, :, f0 : f0 + gw]  # [KD, 128, gw, 128]
            parts.append(blk.transpose(1, 0, 2, 3).reshape(128, KD * gw * 128))
            f0 += gw
        w1r = np.concatenate(parts, axis=1)  # [128, KD*F]
        w2r = (
            W2[e]
            .reshape(NP, W2P, 128, D)
            .transpose(0, 2, 1, 3)
            .reshape(NP, 128, W2P * D)
        )
        packed.append(
            {
                "w1": np.ascontiguousarray(w1r).astype(np_io, copy=False),
                "w2": np.ascontiguousarray(w2r).astype(np_io, copy=False),
                "b1r": np.ascontiguousarray(b1[e].reshape(KF, 128).T),
            }
        )
    _PACK_CACHE[key] = packed
    return packed


def kernel(hidden_states, Wg, W1, b1, W2, b2):
    hidden_states = np.asarray(hidden_states)
    Wg = np.asarray(Wg, dtype=np.float32)
    W1 = np.asarray(W1, dtype=np.float32)
    b1 = np.asarray(b1, dtype=np.float32)
    W2 = np.asarray(W2, dtype=np.float32)
    b2 = np.asarray(b2, dtype=np.float32)

    orig_shape = hidden_states.shape
    D = orig_shape[-1]
    x = np.ascontiguousarray(hidden_states, dtype=np.float32).reshape(-1, D)
    E, _, F = W1.shape
    KD, KF = D // 128, F // 128

    if E != N_CORES or D % 128 != 0 or F % 128 != 0 or KF % W2P != 0:
        return _kernel_numpy(x, Wg, W1, b1, W2, b2).reshape(orig_shape)

    idx, gate = _gate_host(x, Wg)
    order = np.argsort(idx, kind="stable")
    counts = np.bincount(idx, minlength=E)
    starts = np.concatenate([[0], np.cumsum(counts)])

    # Capacity: common padded token count per core. Slab width must be
    # 256..512 (PSUM bank limit / fp32r fast path).
    C = max(256, _roundup(int(counts.max()), 16))
    n_slabs = -(-C // 512)
    C = n_slabs * max(256, _roundup(-(-C // n_slabs), 16))
    CS = C // n_slabs

    mode = MODE
    np_io = np.float32
    if mode == "bf16":
        import ml_dtypes

        np_io = ml_dtypes.bfloat16

    nc = _get_bass(C, n_slabs, mode, D, F)

    wpacked = _pack_weights(W1, b1, W2, np_io)
    in_maps = []
    for e in range(E):
        ids = order[starts[e] : starts[e + 1]]
        xe = np.zeros((C, D), dtype=np.float32)
        xe[: ids.size] = x[ids]
        # [n_slabs, 128, KD*CS] SBUF image (contiguous per partition row)
        xr = xe.reshape(n_slabs, CS, KD, 128).transpose(0, 3, 2, 1)
        xr = np.ascontiguousarray(xr.reshape(n_slabs, 128, KD * CS))
        in_maps.append({"xX": xr.astype(np_io, copy=False), **wpacked[e]})

    res = run_bass_kernel_spmd(nc, in_maps, list(range(N_CORES)))

    out = np.zeros_like(x)
    for e in range(E):
        ids = order[starts[e] : starts[e + 1]]
        if ids.size == 0:
            continue
        yr = np.asarray(res.results[e]["yX"], dtype=np.float32)
        y = yr.reshape(n_slabs, 128, KD, CS).transpose(0, 3, 2, 1).reshape(C, D)
        out[ids] = gate[ids, None] * (y[: ids.size] + b2[e])
    return out.reshape(orig_shape)
